# revision 25
# baseline (speedup 1.0000x reference)
"""Trainium2 Bass kernel for multi-head causal self-attention.

Problem: nn_MultiHeadSelfAttention (B=2, T=2048, D=1024, H=16, hd=64), fp32.

Sharding (8 NeuronCores, single NEFF, SPMD with per-core input slices):
  core c -> batch b = c // 4, heads h0 = (c % 4) * 4 .. h0+4  (4 heads/core).
  Each core computes the QKV projection for its heads, causal flash-style
  attention (upper-triangle blocks skipped; no max-subtraction -- scores are
  O(+-10) so exp is safe in fp32), and a partial output projection over its
  head slice. The host sums the 4 partials per batch and adds b_proj.

All matmuls run in float32r (~1.5e-4 rel err, full rate at free-dim >= 256).
Inputs are DMA'd contiguously and transposed on-chip with PE-transpose (a
strided "transposed" DMA degenerates to 4-byte descriptors). Attention
processes head PAIRS: the two heads' 64-deep QK^T contractions occupy PE row
groups (0,0)/(64,0) and execute concurrently. The t-chunk loop interleaves
QKV projection, attention, and the output projection so the PE stays dense.
"""

import os

import numpy as np

import concourse.bacc as bacc
import concourse.mybir as mybir
import concourse.tile as tile
from concourse import bass_utils
from concourse.bass_interp import get_hw_module
from concourse.masks import make_identity, make_upper_triangular

# Problem constants (hardcoded per contract).
D = 1024
H = 16
HD = 64
B = 2
T = 2048
NCORES = 8
NH = 4          # heads per core
QC = 512        # query-chunk width
NQC = T // QC   # 4
NKT = T // 128  # 16
SM_SCALE = 1.0 / np.sqrt(HD)

F32 = mybir.dt.float32
F32R = mybir.dt.float32r


def _build():
    nc = bacc.Bacc("TRN2", target_bir_lowering=False, debug=False, num_devices=NCORES)

    xT_d = nc.dram_tensor("xT", [D, T], F32, kind="ExternalInput").ap()
    wqkvT_d = nc.dram_tensor("wqkvT", [D, 3 * NH * HD], F32, kind="ExternalInput").ap()
    bqkv_d = nc.dram_tensor("bqkv", [3 * NH * HD], F32, kind="ExternalInput").ap()
    wprojT_d = nc.dram_tensor("wprojT", [NH * HD, D], F32, kind="ExternalInput").ap()
    z_d = nc.dram_tensor("zpart", [T, D], F32, kind="ExternalOutput").ap()

    with tile.TileContext(nc) as tc:
        with (
            tc.tile_pool(name="persist", bufs=1) as pp,
            tc.tile_pool(name="xt", bufs=2) as xtp,
            tc.tile_pool(name="pt", bufs=4) as ptp,
            tc.tile_pool(name="sbtmp", bufs=3) as sbtmp,
            tc.tile_pool(name="zout", bufs=2) as zoutp,
            tc.tile_pool(name="st", bufs=2, space="PSUM") as stp,
            tc.tile_pool(name="yaug", bufs=2, space="PSUM") as yaugp,
            tc.tile_pool(name="gen", bufs=2, space="PSUM") as genp,
        ):
            # ---- constants ----
            ones_f32 = pp.tile([128, 128], F32, tag="ones_f32")
            nc.vector.memset(ones_f32[:], 1.0)
            zeros_f32 = pp.tile([128, 128], F32, tag="zeros_f32")
            nc.vector.memset(zeros_f32[:], 0.0)
            ident = pp.tile([128, 128], F32, tag="ident")
            make_identity(nc, ident[:])
            mask01 = pp.tile([128, 128], F32, tag="mask01")
            make_upper_triangular(nc, mask01[:], val=1.0, diag=True)
            maskz = pp.tile([128, 256], F32, tag="maskz")  # [zeros | upper-tri]
            nc.vector.memset(maskz[:, 0:128], 0.0)
            nc.vector.tensor_copy(maskz[:, 128:256], mask01[:])

            # ---- PE warmup: dense dummy matmuls while initial DMAs land ----
            warm = pp.tile([128, 512], F32R, tag="warm")
            for i in range(4):
                nc.vector.tensor_copy(
                    warm[:, i * 128:(i + 1) * 128], zeros_f32[:]
                )
            for i in range(16):
                wps = stp.tile([128, 512], F32, tag="st", name=f"warmps{i}")
                nc.tensor.matmul(
                    wps[:], warm[:, 0:128], warm[:], start=True, stop=True
                )

            # ---- persistent tensors ----
            wq_t = pp.tile([128, 8 * 256], F32R, tag="wq_t")
            wk_t = pp.tile([128, 8 * 256], F32R, tag="wk_t")
            wv_t = pp.tile([128, 8 * 256], F32R, tag="wv_t")
            wp_t = pp.tile([128, 2 * 1024], F32R, tag="wp_t")
            qt_sb = [pp.tile([128, T], F32R, tag=f"qt{i}", name=f"qt{i}") for i in range(2)]
            kt_sb = [pp.tile([128, T], F32R, tag=f"kt{i}", name=f"kt{i}") for i in range(2)]
            vaug = pp.tile([128, NKT * 260], F32R, tag="vaug")
            y_all = pp.tile([128, NKT * 256], F32, tag="y_all")
            ysb = [pp.tile([128, T], F32R, tag=f"ysb{i}", name=f"ysb{i}") for i in range(2)]

            bias_sb = pp.tile([128, 4], F32, tag="bias")  # q e-tiles 0,1; k 2,3
            for i in range(4):
                nc.sync.dma_start(
                    bias_sb[:, i:i + 1],
                    bqkv_d[i * 128:(i + 1) * 128].rearrange("(e o) -> e o", o=1),
                )
            bv_row = pp.tile([1, 256], F32, tag="bv_row")
            nc.sync.dma_start(
                bv_row[:], bqkv_d[512:768].rearrange("(o e) -> o e", o=1)
            )
            bv_bc = pp.tile([128, 256], F32, tag="bv_bc")
            nc.gpsimd.partition_broadcast(bv_bc[:], bv_row[:])

            def qkv_units(tcn):
                """QKV projection for t-chunk tcn as a list of work units.

                xt_all layout: [d-partition(128) x (kc(8) * t(512))].
                """
                ctx = {}

                def u_dma():
                    xt_all = xtp.tile(
                        [128, 8 * QC], F32R, tag="xt", name=f"xt{tcn}"
                    )
                    xv = xt_all[:].rearrange("p (kc t) -> p kc t", t=QC)
                    ctx["xv"] = xv
                    for kc in range(8):
                        nc.sync.dma_start(
                            xv[:, kc, :],
                            xT_d[kc * 128:(kc + 1) * 128,
                                 tcn * QC:(tcn + 1) * QC].bitcast(F32R),
                        )

                def mk_qk(w_t, dst, bcol, e):
                    def u():
                        xv = ctx["xv"]
                        ps = genp.tile([128, QC], F32, tag="gen")
                        for kc in range(8):
                            nc.tensor.matmul(
                                ps[:],
                                w_t[:, kc * 256 + e * 128:kc * 256 + (e + 1) * 128],
                                xv[:, kc, :],
                                start=(kc == 0),
                                stop=(kc == 7),
                            )
                        nc.vector.tensor_scalar_add(
                            dst[e][:, tcn * QC:(tcn + 1) * QC],
                            ps[:],
                            bias_sb[:, bcol + e:bcol + e + 1],
                        )
                    return u

                def mk_v(tti):
                    def u():
                        xv = ctx["xv"]
                        tt = tcn * 4 + tti
                        ps = genp.tile([128, 256], F32, tag="gen")
                        for kc in range(8):
                            nc.tensor.matmul(
                                ps[:],
                                xv[:, kc, tti * 128:(tti + 1) * 128],
                                wv_t[:, kc * 256:(kc + 1) * 256],
                                start=(kc == 0),
                                stop=(kc == 7),
                            )
                        seg = vaug[:, tt * 260:(tt + 1) * 260].rearrange(
                            "p (h c) -> p h c", c=65
                        )
                        nc.vector.scalar_tensor_tensor(
                            seg[:, :, 0:64],
                            ps[:].rearrange("p (h c) -> p h c", c=64),
                            1.0,
                            bv_bc[:].rearrange("p (h c) -> p h c", c=64),
                            op0=mybir.AluOpType.mult,
                            op1=mybir.AluOpType.add,
                        )
                        nc.vector.tensor_copy(
                            seg[:, :, 64:65],
                            ones_f32[:, 0:4].rearrange("p (h c) -> p h c", c=1),
                        )
                    return u

                units = [u_dma]
                for w_t, dst, bcol in ((wq_t, qt_sb, 0), (wk_t, kt_sb, 2)):
                    for e in range(2):
                        units.append(mk_qk(w_t, dst, bcol, e))
                for tti in range(4):
                    units.append(mk_v(tti))
                return units

            def attention2(hp, qc, tick=None):
                """Head pair (2*hp, 2*hp+1) x one query chunk.

                S^T for both heads lands in one [128, 1024] psum tile
                (cols 0:512 even head / 512:1024 odd head); the two QK^T
                matmuls use PE row groups (0,0) and (64,0) concurrently.
                """
                qth = qt_sb[hp]
                kth = kt_sb[hp]
                he, ho = 2 * hp, 2 * hp + 1
                nkt = 4 * qc + 4
                ya_e = yaugp.tile([65, QC], F32, tag="yaug", name=f"yae{hp}_{qc}")
                ya_o = yaugp.tile([65, QC], F32, tag="yaug", name=f"yao{hp}_{qc}")
                def st_pair(kti):
                    d0 = kti * 128 - qc * QC  # k0 - q0
                    f0 = 256 if d0 >= 256 else (128 if d0 == 128 else 0)
                    st = stp.tile([128, 2 * QC], F32, tag="st", name=f"st{kti}")
                    pt = ptp.tile([128, 2 * QC], F32R, tag="pt", name=f"pt{kti}")
                    for half, po in ((0, 0), (1, 64)):
                        nc.tensor.matmul(
                            st[:, half * QC + f0:(half + 1) * QC],
                            kth[po:po + 64, kti * 128:(kti + 1) * 128],
                            qth[po:po + 64, qc * QC + f0:(qc + 1) * QC],
                            start=True,
                            stop=True,
                        )
                    if f0 == 0:
                        nc.scalar.activation(
                            pt[:], st[:],
                            mybir.ActivationFunctionType.Exp,
                            scale=float(SM_SCALE),
                        )
                    else:
                        stv = st[:].rearrange("p (j c) -> p j c", c=QC)
                        ptv = pt[:].rearrange("p (j c) -> p j c", c=QC)
                        nc.scalar.activation(
                            ptv[:, :, f0:QC], stv[:, :, f0:QC],
                            mybir.ActivationFunctionType.Exp,
                            scale=float(SM_SCALE),
                        )
                    return pt

                def av_pair(kti, pt):
                    d0 = kti * 128 - qc * QC
                    f0 = 256 if d0 >= 256 else (128 if d0 == 128 else 0)
                    if d0 >= 0:
                        for half in range(2):
                            if d0 > f0:
                                nc.vector.tensor_mul(
                                    pt[:, half * QC + f0:half * QC + d0 + 128],
                                    pt[:, half * QC + f0:half * QC + d0 + 128],
                                    maskz[:],
                                )
                            else:
                                nc.vector.tensor_mul(
                                    pt[:, half * QC + d0:half * QC + d0 + 128],
                                    pt[:, half * QC + d0:half * QC + d0 + 128],
                                    mask01[:],
                                )
                    for ya, h, half in ((ya_e, he, 0), (ya_o, ho, 1)):
                        nc.tensor.matmul(
                            ya[0:65, f0:QC],
                            vaug[:, kti * 260 + h * 65:kti * 260 + (h + 1) * 65],
                            pt[:, half * QC + f0:(half + 1) * QC],
                            start=(kti == 0),
                            stop=(kti == nkt - 1),
                        )

                prev = None
                for kti in range(nkt):
                    pt = st_pair(kti)
                    if prev is not None:
                        av_pair(prev[0], prev[1])
                        if tick is not None:
                            tick()
                    prev = (kti, pt)
                av_pair(prev[0], prev[1])
                if tick is not None:
                    tick()
                # transpose y_aug^T back to [q, 65]; normalize by row-sums
                for ya, h in ((ya_e, he), (ya_o, ho)):
                    ya_sb = sbtmp.tile([65, QC], F32, tag="ya_sb")
                    nc.vector.tensor_copy(ya_sb[:], ya[:])
                    for sub in range(4):
                        t1 = yaugp.tile([128, 65], F32, tag="yaug")
                        nc.tensor.matmul(
                            t1[:],
                            ya_sb[0:65, sub * 128:(sub + 1) * 128],
                            ident[0:65, 0:65],
                            is_transpose=True,
                        )
                        rec = sbtmp.tile([128, 1], F32, tag="rec")
                        nc.vector.reciprocal(rec[:], t1[:, 64:65])
                        tt = qc * 4 + sub
                        nc.vector.tensor_scalar_mul(
                            y_all[:, tt * 256 + h * 64:tt * 256 + (h + 1) * 64],
                            t1[:, 0:64],
                            rec[:],
                        )

            def proj(tt):
                """Output projection for one t-tile: y_all -> y^T -> z."""
                for ci in range(2):
                    t2 = genp.tile([128, 128], F32, tag="gen")
                    nc.tensor.matmul(
                        t2[:],
                        y_all[:, tt * 256 + ci * 128:tt * 256 + (ci + 1) * 128],
                        ident[:],
                        is_transpose=True,
                    )
                    nc.vector.tensor_copy(ysb[ci][:, tt * 128:(tt + 1) * 128], t2[:])
                for dc in range(2):
                    zp = genp.tile([128, QC], F32, tag="gen")
                    for ci in range(2):
                        nc.tensor.matmul(
                            zp[:],
                            ysb[ci][:, tt * 128:(tt + 1) * 128],
                            wp_t[:, ci * 1024 + dc * QC:ci * 1024 + (dc + 1) * QC],
                            start=(ci == 0),
                            stop=(ci == 1),
                        )
                    zs = zoutp.tile([128, QC], F32, tag="zs")
                    if dc == 0:
                        nc.scalar.copy(zs[:], zp[:])
                    else:
                        nc.vector.tensor_copy(zs[:], zp[:])
                    nc.sync.dma_start(
                        z_d[tt * 128:(tt + 1) * 128, dc * QC:(dc + 1) * QC], zs[:]
                    )

            # ---- interleaved schedule with one global filler queue ----
            # attention(qc)'s kt only needs K/V chunk kt//4, so later
            # chunks' QKV units can drip in across chunk boundaries; Tile's
            # dependency tracking inserts any residual waits.
            units0 = qkv_units(0)
            units0[0]()  # x chunk 0 DMA first -- the PE's first dependency
            for col0, wt_dst in ((0, wq_t), (256, wk_t)):
                for kc in range(8):
                    nc.sync.dma_start(
                        wt_dst[:, kc * 256:(kc + 1) * 256],
                        wqkvT_d[kc * 128:(kc + 1) * 128,
                                col0:col0 + 256].bitcast(F32R),
                    )
            for kc in range(8):
                nc.sync.dma_start(
                    wv_t[:, kc * 256:(kc + 1) * 256],
                    wqkvT_d[kc * 128:(kc + 1) * 128, 512:768].bitcast(F32R),
                )
            for u in units0[1:]:
                u()
            for ci in range(2):
                nc.sync.dma_start(
                    wp_t[:, ci * 1024:(ci + 1) * 1024],
                    wprojT_d[ci * 128:(ci + 1) * 128, :].bitcast(F32R),
                )
            queue = []
            for t in range(1, NQC):
                queue.extend(qkv_units(t))
            total_ticks = sum(2 * (4 * t + 4) for t in range(NQC))
            state = {"tick": 0, "emitted": 0}

            def tick():
                state["tick"] += 1
                # proj units for finished chunks join the queue lazily (in
                # attention2 below); pace emission evenly across all ticks.
                target = (state["tick"] * 39) // total_ticks + 1
                while queue and state["emitted"] < target:
                    queue.pop(0)()
                    state["emitted"] += 1

            for tcn in range(NQC):
                for hp in range(2):
                    attention2(hp, qc=tcn, tick=tick)
                if tcn > 0:
                    for i in range(4):
                        queue.append(
                            (lambda t: lambda: proj(t))(tcn * 4 - 4 + i)
                        )
            while queue:
                queue.pop(0)()
            for tti in range(4):
                proj(12 + tti)

    nc.compile()
    nc.m = get_hw_module(nc.m)
    return nc


_NC_CACHE = None


def _get_nc():
    global _NC_CACHE
    if _NC_CACHE is None:
        _NC_CACHE = _build()
    return _NC_CACHE


def _in_maps(x, w_qkv, b_qkv, w_proj):
    x = np.asarray(x, dtype=np.float32)
    w_qkv = np.asarray(w_qkv, dtype=np.float32)
    b_qkv = np.asarray(b_qkv, dtype=np.float32)
    w_proj = np.asarray(w_proj, dtype=np.float32)
    maps = []
    for c in range(NCORES):
        b = c // 4
        h0 = (c % 4) * NH
        r0 = h0 * HD
        rows = np.r_[r0:r0 + 256, D + r0:D + r0 + 256, 2 * D + r0:2 * D + r0 + 256]
        maps.append(
            {
                "xT": np.ascontiguousarray(x[b].T),
                "wqkvT": np.ascontiguousarray(w_qkv[rows].T),
                "bqkv": np.ascontiguousarray(b_qkv[rows]),
                "wprojT": np.ascontiguousarray(w_proj[:, r0:r0 + 256].T),
            }
        )
    return maps


def _run(inputs, trace=False, **kw):
    nc = _get_nc()
    maps = _in_maps(
        inputs["x"], inputs["w_qkv"], inputs["b_qkv"], inputs["w_proj"]
    )
    return bass_utils.run_bass_kernel_spmd(
        nc, maps, core_ids=list(range(NCORES)), trace=trace, **kw
    )


def kernel(x, attn_mask, w_qkv, b_qkv, w_proj, b_proj):
    # attn_mask is the fixed causal (lower-triangular) mask; causality is
    # implemented structurally in the kernel.
    res = _run(
        {"x": x, "w_qkv": w_qkv, "b_qkv": b_qkv, "w_proj": w_proj}
    )
    out = np.zeros((B, T, D), dtype=np.float32)
    for c in range(NCORES):
        out[c // 4] += res.results[c]["zpart"]
    out += np.asarray(b_proj, dtype=np.float32)
    return out
